# revision 1
# baseline (speedup 1.0000x reference)
"""Trainium2 Bass kernel for AffinityLoss (nn_AffinityLoss_70875550318911), v5.

Math: loss = mean over (n, a, b, l) of BCEWithLogits(aff_map, lb_map) where
aff_map[n,a,b,l] = sum_c lu[n,c,a,l]*lu[n,c,b,l] over 3x3 unfold positions a,b.

Reformulation: pairs (a,b) sharing relative offset d=(di,dj) share one
correlation map D_d[p] = sum_c logits[c,p]*logits[c,p+d]; by symmetry only 13
offsets are needed. Border multiplicities factorize into row weights rw(y)
times col weights cw(x). With u = sigmoid(-D):

  contrib_d = sum_{r,x} (-rw*cw) * ln(u)  +  (-rw*cw*m) * D   (m = label match)
  loss = sum_d sym_d * contrib_d / (n * 81 * 382^2)

Engine split (per core: 96 owned image rows = 2 batches x 48):
  layout: 114 partitions = (c=19, group=6), free = 18 rows x 384 (16 owned
  + 2 halo), bf16.
  - DVE:    shifted products (one 6144-elem TT per offset, 2x mode) for 10
            offsets, plus 26 small TTs j = val * weight (weights host-folded)
  - ACT:    offset (0,0)'s product as Square(L); Dc = copy(D), u =
            sigmoid(-D), lt = ln(u); sigmoid/ln batched into phase-pairs
  - Pool:   products of the AFF_POOL_OFFS offsets, scheduled at positions
            where the in-order PE never waits on them
  - PE:     c-sum as 16 accumulating matmuls per offset against a sliding
            0/1 indicator [114, 96] -> D [96 rows, 384] f32 in PSUM; plus 26
            ones-vector matmuls accumulating sum_r j into PT [1, 384]

Host sums PT over x and cores and applies the global scale.
"""
import os
import numpy as np
import ml_dtypes

NCORES = 8
N, C, H, W = 2, 19, 384, 384
KS = 3
BAND = H // NCORES            # 48 owned rows per core per batch
NGRP = 6                      # groups: (batch=2) x (row-block=3)
GR = 16                       # owned rows per group
TRG = GR + 2                  # rows stored per group (owned + halo)
PART = C * NGRP               # 114 partitions
FREE = TRG * W                # 6912 data elems per partition
PADF = FREE + 4               # +2 pad each side
OROWS = NGRP * GR             # 96 output rows (partitions of D)
MULF = GR * W                 # 6144 elems per offset multiply
NOFF = 13
IW = OROWS + (GR - 1) * NGRP  # 186 indicator columns

# (di, dj, sym): di >= 0; for di == 0 only dj >= 0. sym 2 covers (-di,-dj).
OFFSETS = [(0, 0, 1.0), (0, 1, 2.0), (0, 2, 2.0),
           (1, -2, 2.0), (1, -1, 2.0), (1, 0, 2.0), (1, 1, 2.0), (1, 2, 2.0),
           (2, -2, 2.0), (2, -1, 2.0), (2, 0, 2.0), (2, 1, 2.0), (2, 2, 2.0)]

# offset -> (emission position, engine): q0's product runs as Square on ACT;
# POOL_OFFS products run on Pool at positions late enough that the in-order
# PE has already caught up with their (slow) production
POOL_OFFS = [int(x) for x in
             os.environ.get("AFF_POOL_OFFS", "5,10").split(",") if x]
POOL_POS = [int(x) for x in
            os.environ.get("AFF_POOL_POS", "6,9").split(",") if x]
PHASES = [int(x) for x in os.environ.get("AFF_PHASES", "5,9,13").split(",")]

BF16 = ml_dtypes.bfloat16

_PROGRAM = None
LAST_RESULTS = None  # BassKernelResults of the most recent run (for profiling)


def _mult_weight(d: int, p: int, size: int = H) -> int:
    """Number of 3x3 window anchors pairing pixel p with p+d along one axis."""
    lo, hi = max(0, -d), 2 - max(d, 0)
    lo2, hi2 = max(lo, p - (size - KS)), min(hi, p)
    return max(0, hi2 - lo2 + 1)


def _build_program():
    import concourse.tile as tile
    from concourse import bacc, mybir
    from concourse.alu_op_type import AluOpType
    from contextlib import ExitStack

    bf = mybir.dt.bfloat16
    f32 = mybir.dt.float32
    A = AluOpType
    AF = mybir.ActivationFunctionType

    nc = bacc.Bacc("TRN2", target_bir_lowering=False, debug=False,
                   num_devices=NCORES)

    lg_d = nc.dram_tensor("lg", [PART, PADF], bf, kind="ExternalInput")
    wts_d = nc.dram_tensor("wts", [OROWS, 2 * NOFF * W], bf,
                           kind="ExternalInput")
    ind_d = nc.dram_tensor("ind", [PART, IW], bf, kind="ExternalInput")
    out = nc.dram_tensor("out", [1, W], f32, kind="ExternalOutput")

    with ExitStack() as ctx:
        tc = ctx.enter_context(tile.TileContext(nc))
        singles = ctx.enter_context(tc.tile_pool(name="singles", bufs=1))
        work = ctx.enter_context(tc.tile_pool(
            name="work", bufs=int(os.environ.get("AFF_WORK_BUFS", "4"))))
        pipe = ctx.enter_context(tc.tile_pool(
            name="pipe", bufs=int(os.environ.get("AFF_PIPE_BUFS", "6"))))
        psum = ctx.enter_context(tc.tile_pool(
            name="psum", bufs=int(os.environ.get("AFF_PSUM_BUFS", "4")),
            space="PSUM"))
        psum1 = ctx.enter_context(tc.tile_pool(name="psum1", bufs=1,
                                               space="PSUM"))

        LG = singles.tile([PART, PADF], bf, name="LG")
        WTS = singles.tile([OROWS, 2 * NOFF * W], bf, name="WTS")
        MCW = [WTS[:, q * W:(q + 1) * W] for q in range(NOFF)]
        CWT = [WTS[:, (NOFF + q) * W:(NOFF + q + 1) * W]
               for q in range(NOFF)]
        IND = singles.tile([PART, IW], bf)
        ONES = singles.tile([OROWS, 1], bf)
        UT = [singles.tile([OROWS, W], bf, name=f"U{q}") for q in range(NOFF)]
        DC = [singles.tile([OROWS, W], bf, name=f"DC{q}")
              for q in range(NOFF)]
        LT = [singles.tile([OROWS, W], bf, name=f"LT{q}")
              for q in range(NOFF)]

        nc.vector.memset(ONES[:], 1.0)
        from concourse.tile import add_dep_helper
        warm = []

        # logits band in 4 pieces; first piece dispatched before IND so the
        # first product chunk starts ASAP; weights go via the Pool SWDGE
        # queue to keep them off the serial HWDGE dispatch path
        splits = [0, 2 + 2 * W, 2 + 8 * W, 2 + 13 * W, PADF]
        queues = [nc.sync, nc.scalar, nc.scalar, nc.sync]
        pieces = list(zip(queues, zip(splits[:-1], splits[1:])))
        qd, (lo, hi) = pieces[0]
        qd.dma_start(LG[:, lo:hi], lg_d[:, lo:hi])
        nc.sync.dma_start(IND[:], ind_d[:])
        for qd, (lo, hi) in pieces[1:]:
            qd.dma_start(LG[:, lo:hi], lg_d[:, lo:hi])
        nc.sync.dma_start(WTS[:, 0:NOFF * W], wts_d[:, 0:NOFF * W])
        nc.sync.dma_start(WTS[:, NOFF * W:], wts_d[:, NOFF * W:])

        # emission sequence: q0 (ACT-Square product) first, pool offsets at
        # POOL_POS, the rest on DVE in offset order
        seq = [q for q in range(NOFF) if q != 0 and q not in POOL_OFFS]
        seq.insert(int(os.environ.get("AFF_SQPOS", "0")), 0)
        for p, q in sorted(zip(POOL_POS, POOL_OFFS)):
            seq.insert(p, q)
        assert sorted(seq) == list(range(NOFF))

        pool_prods = {}
        act_seq = []  # ACT instrs chained in emission order so the scheduler
        # can't interleave sigmoid-table and ln-table phases

        def _act(*args, **kw):
            inst = nc.scalar.activation(*args, **kw)
            act_seq.append(inst)
            return inst

        def emit_pool_prods():
            first = True
            for q in POOL_OFFS:
                di, dj, _sym = OFFSETS[q]
                shift = di * W + dj
                pp = singles.tile([PART, MULF], bf, name=f"poolprod{q}")
                chunks = ([(0, 5 * W), (5 * W, 11 * W), (11 * W, MULF)]
                          if first else [(0, MULF)])
                for lo, hi in chunks:
                    nc.gpsimd.tensor_tensor(
                        pp[:, lo:hi], LG[:, 2 + lo:2 + hi],
                        LG[:, 2 + shift + lo:2 + shift + hi], A.mult)
                pool_prods[q] = pp
                first = False

        PT = psum1.tile([1, W], f32)
        emat = [0]

        def e_matmul(t):
            nc.tensor.matmul(PT[:], ONES[:], t[:],
                             start=(emat[0] == 0), stop=(emat[0] == 2 * NOFF - 1),
                             skip_group_check=True)
            emat[0] += 1

        jt = {}

        def emit_j0(q, eng=None):
            j0 = pipe.tile([OROWS, W], bf, tag="j0")
            (eng or nc.vector).tensor_tensor(j0[:], DC[q][:], MCW[q], A.mult)
            jt[(0, q)] = j0

        def emit_j1(q):
            j1 = pipe.tile([OROWS, W], bf, tag="j1")
            nc.vector.tensor_tensor(j1[:], LT[q][:], CWT[q], A.mult)
            jt[(1, q)] = j1

        def phase_a(pos, q):
            di, dj, _sym = OFFSETS[q]
            shift = di * W + dj

            if q == 0:
                # L * L = Square(L) on the ACT engine, chunked on DMA pieces
                prod = work.tile([PART, MULF], bf, tag="prodsq")
                for lo, hi in [(0, 2 * W), (2 * W, 6 * W), (6 * W, 11 * W),
                               (11 * W, MULF)]:
                    _act(prod[:, lo:hi], LG[:, 2 + lo:2 + hi], AF.Square)
            elif q in pool_prods:
                prod = pool_prods[q]
            else:
                prod = work.tile([PART, MULF], bf, tag="prod")
                if pos <= 2:
                    chunks = [(0, 2 * W - 8), (2 * W - 8, 6 * W),
                              (6 * W, 11 * W), (11 * W, MULF)]
                elif pos <= 4:
                    chunks = [(0, 6 * W), (6 * W, 11 * W), (11 * W, MULF)]
                else:
                    chunks = [(0, MULF)]
                for lo, hi in chunks:
                    nc.vector.tensor_tensor(
                        prod[:, lo:hi], LG[:, 2 + lo:2 + hi],
                        LG[:, 2 + shift + lo:2 + shift + hi], A.mult)

            D = psum.tile([OROWS, W], f32, tag="D")
            for s in range(GR):
                dm = nc.tensor.matmul(
                    D[:], IND[:, (GR - 1) * NGRP - NGRP * s:
                              (GR - 1) * NGRP - NGRP * s + OROWS],
                    prod[:, s * W:(s + 1) * W],
                    start=(s == 0), stop=(s == GR - 1))
                if warm and pos == 0 and s == 0:
                    add_dep_helper(dm.ins, warm[-1].ins, sync=False,
                                   reason="after warmup")

            _act(DC[q][:], D[:], AF.Copy)
            _act(UT[q][:], D[:], AF.Sigmoid, scale=-1.0)

        def phase_b(q):
            _act(LT[q][:], UT[q][:], AF.Ln)

        pairs = []
        lo = 0
        for hi in PHASES:
            pairs.append([seq[pos] for pos in range(lo, hi)])
            lo = hi

        done_j0, done_j1, edone, ready_e = [], [], [], []
        for pi, members in enumerate(pairs):
            for i, q in enumerate(members):
                phase_a(sum(len(p) for p in pairs[:pi]) + i, q)
            # after this pair's A-emission: j0-TTs of pair-1, j1s of pair-2
            if pi >= 1:
                for q in pairs[pi - 1]:
                    emit_j0(q, nc.gpsimd if pi == 1 else None)
                    done_j0.append(q)
            if pi >= 2:
                for q in pairs[pi - 2]:
                    emit_j1(q)
                    done_j1.append(q)
            if os.environ.get("AFF_E_END", "0") != "1":
                for key in sorted(jt):
                    if key not in edone and key[1] in [x for p in pairs[:max(pi - 1, 0)] for x in p]:
                        e_matmul(jt[key])
                        edone.append(key)
            for q in members:
                phase_b(q)
            if pi == 0:
                emit_pool_prods()
        for q in (q for p in pairs for q in p):
            if q not in done_j0:
                emit_j0(q)
        for q in (q for p in pairs for q in p):
            if q not in done_j1:
                emit_j1(q)
        for key in sorted(jt):
            if key not in edone:
                e_matmul(jt[key])
                edone.append(key)

        for i in range(1, len(act_seq)):
            add_dep_helper(act_seq[i].ins, act_seq[i - 1].ins, sync=False,
                           reason="ACT emission order (table-set phases)")

        res = singles.tile([1, W], f32)
        nc.vector.tensor_copy(res[:], PT[:])
        nc.sync.dma_start(out[:], res[:])
    nc.compile()
    return nc


def _host_inputs(logits: np.ndarray, labels: np.ndarray):
    logits = np.asarray(logits, dtype=np.float32)
    labels = np.asarray(labels)
    lg_bf = logits.astype(BF16)                      # (n, c, h, w)

    cw = np.zeros((5, W), dtype=np.float32)
    for j, dj in enumerate(range(-2, 3)):
        cw[j] = [_mult_weight(dj, px, W) for px in range(W)]
    wy_tab = np.array([[_mult_weight(d, py, H) for py in range(H)]
                      for d in range(3)], dtype=np.float32)

    ind = np.zeros((PART, IW), dtype=BF16)
    ind[np.arange(PART), (GR - 1) * NGRP + np.arange(PART) % NGRP] = 1.0

    in_maps = []
    for k in range(NCORES):
        m = {"ind": ind}
        # logits band: [c*6+g, 2 + r*384 + x], g = b*3+gb,
        # rows y = k*48 + gb*16 + r for r in 0..17 (zero-padded past H)
        ga = np.zeros((PART, PADF), dtype=BF16)
        for b in range(N):
            for gb in range(3):
                g = b * 3 + gb
                y0 = k * BAND + gb * GR
                rows = min(TRG, H - y0)
                blk = np.zeros((C, TRG, W), dtype=BF16)
                blk[:, :rows, :] = lg_bf[b, :, y0:y0 + rows, :]
                ga[g::NGRP, 2:2 + FREE] = blk.reshape(C, FREE)
        m["lg"] = ga

        # D partition 6s+g <-> (batch b, image row y = k*48 + gb*16 + s)
        rw = np.zeros((OROWS, NOFF), dtype=np.float32)
        for q, (di, dj, sym) in enumerate(OFFSETS):
            for g in range(NGRP):
                b, gb = divmod(g, 3)
                ys = k * BAND + gb * GR + np.arange(GR)
                rw[g::NGRP, q] = sym * wy_tab[di, ys]

        wts = np.zeros((OROWS, 2 * NOFF * W), dtype=np.float32)
        for q, (di, dj, sym) in enumerate(OFFSETS):
            wts[:, (NOFF + q) * W:(NOFF + q + 1) * W] = \
                rw[:, q:q + 1] * -cw[dj + 2]

        # mcw_q = -(rw * cw * [labels match]) in the permuted row order
        for q, (di, dj, sym) in enumerate(OFFSETS):
            mc = np.zeros((OROWS, W), dtype=np.float32)
            x0, x1 = max(0, -dj), W - max(dj, 0)
            for g in range(NGRP):
                b, gb = divmod(g, 3)
                ys = k * BAND + gb * GR + np.arange(GR)
                val = ys + di < H
                yv = ys[val]
                mm = (labels[b, yv, x0:x1] == labels[b, yv + di, x0 + dj:x1 + dj])
                blk = np.zeros((GR, W), dtype=np.float32)
                blk[val, x0:x1] = -(mm * cw[dj + 2][x0:x1])
                mc[g::NGRP] = blk
            wts[:, q * W:(q + 1) * W] = mc * rw[:, q:q + 1]
        m["wts"] = wts.astype(BF16)
        in_maps.append(m)
    return in_maps


def kernel(logits: np.ndarray, labels: np.ndarray) -> np.ndarray:
    global _PROGRAM, LAST_RESULTS
    from concourse.bass_utils import run_bass_kernel_spmd

    if _PROGRAM is None:
        _PROGRAM = _build_program()

    in_maps = _host_inputs(logits, labels)
    trace = bool(int(os.environ.get("AFF_TRACE", "0")))
    results = run_bass_kernel_spmd(
        _PROGRAM, in_maps, core_ids=list(range(NCORES)), trace=trace)
    LAST_RESULTS = results

    total = 0.0
    for r in results.results:
        total += float(np.asarray(r["out"], dtype=np.float64).sum())
    Lwin = (H - KS + 1) * (W - KS + 1)
    return np.float32(total / (N * KS**4 * Lwin))

